# revision 9
# baseline (speedup 1.0000x reference)
"""Trainium2 Bass kernel for nn_DemandPredictionModel (2-layer GCN + time MLP).

v3 strategy (8 NeuronCores, SPMD single program).

Key algebraic move (v2): the final projection is linear with OUT_CH=1, so
layer 2 collapses to a scalar SpMV:
    x_gcn @ Wf[:256] = A_hat @ (h @ (W2 @ Wf[:256]))
Only ONE full 256-channel gather layer remains; layer 2 needs one fp16
scalar per node.

Pipelined AllGather (v3): the xws table AllGather is split into 4
block-aligned shard-quarters (measured: 4 chunked AGs cost the same total
as one big AG, so the first quarter lands ~3x earlier).  Layer-1 edge
aggregation runs as 4 passes, pass q starting as soon as quarter-table q
arrives; partial block sums are stashed to SBUF in fp16 between passes and
merged back into PSUM with an indself identity matmul.

  - Stage 1: core c owns rows [c*6250, (c+1)*6250).  xws = dinv * (x @ W1)
    (bf16 matmul) -> ag_in_q; AG-q fires as soon as its blocks are written.
    x_time head contribution xtc = relu(tf@Wt+bt) . wf2 (DVE) hides in the
    AG shadow.
  - Pass q: edges with src%6250 in quarter q are gathered by src row via
    SWDGE dma_gather (int16 idx into the [<=13312, 256] quarter table),
    scatter-added into PSUM with [128e x 128d] fp8 indicator matmuls.
    Self-loop stream reads ag_in_q with plain strided DMA (last pass).
    Epilogue per block: h = relu(dinv*agg); z = dinv * (h . w2f) in fp16.
  - z AllGather: [128, 49] fp16 per core (25KB) -> [1024, 49] shared; the
    full z table then lives in SBUF as [128, 8, 49] fp16 (100KB).
  - Stage 4 (scalar SpMV out = A_hat z): edges sorted by padded src id,
    window-aligned <=128-edge tiles (window = one column of the z table).
    Per tile: VALS[e] = SEL_t^T @ zwin (fp8 matmul), rhs[e,b] =
    VALS[e]*BMASK[e,b] (one batched stride-0-broadcast DVE mult per 16-tile
    group), OUT[p,b] += IND_t^T @ rhs (fp8 matmul, single PSUM accumulator).
  - Head: out = dinv*OUT + xtc (+ const), [128, 49] fp32 per core.
"""

import sys

if "/opt/trn_rl_repo" not in sys.path:
    sys.path.insert(0, "/opt/trn_rl_repo")

import numpy as np
import ml_dtypes

import concourse.bacc as bacc
import concourse.bass as bass
import concourse.mybir as mybir
import concourse.tile as tile
from concourse import library_config
from concourse.bass_utils import run_bass_kernel_spmd
from concourse._compat import cdiv

N_NODES = 50000
N_EDGES = 800000
CH = 256
NCORES = 8
RPC = N_NODES // NCORES            # 6250 rows per core
NBLK = cdiv(RPC, 128)              # 49 blocks per core (last has 106 rows)
RPC_PAD = NBLK * 128               # 6272
NWIN = NCORES * NBLK               # 392 z-windows of 128 padded ids
NQ = 4                             # AllGather shard-quarters
QBLK = [0, 13, 25, 37, 49]         # block-aligned quarter boundaries
QROW = [0, 1664, 3200, 4736, 6250]  # row boundaries (min(128*qb, RPC))
CHUNK_TILES = 8                    # L1 gather chunk (x128 edges); SWDGE ring
                                   # caps dma_gather at ~1024 idxs
NQUEUES = 4                        # cycle gathers over all 4 SWDGE queues
SC = 16                            # stage-4 tiles per pipeline group

F8 = ml_dtypes.float8_e4m3
BF16 = ml_dtypes.bfloat16

_cache = {}


def cdiv_arr(a, b):
    return -(-a // b)


def _preprocess(edge_index):
    """Partition/sort edges; build L1 q-streams and stage-4 tile arrays."""
    ei = np.asarray(edge_index)
    src_all = np.concatenate([ei[0], np.arange(N_NODES, dtype=np.int64)])
    dst_all = np.concatenate([ei[1], np.arange(N_NODES, dtype=np.int64)])
    deg = np.bincount(dst_all, minlength=N_NODES).astype(np.float64)
    dinv = (1.0 / np.sqrt(deg)).astype(np.float32)
    sqdeg = np.sqrt(deg).astype(np.float32)

    # ---- L1 q-bucket streams (self loops via dedicated per-block stream) ---
    e_src, e_dst = ei[0], ei[1]
    core = e_dst // RPC
    rel = e_dst - core * RPC
    blk = rel >> 7
    srel = e_src % RPC
    q = np.searchsorted(np.asarray(QROW), srel, side="right") - 1
    qsize = np.diff(np.asarray(QROW))
    # row in quarter table q: shard-major
    tbl_row = (e_src // RPC) * qsize[q] + (srel - np.asarray(QROW)[q])

    # keep the (random) input order within each group — sorted-by-src gathers
    # alias HBM channels and run ~3x slower (measured in v1).
    key = (core * NBLK + blk) * NQ + q
    order = np.argsort(key, kind="stable")
    g_row = tbl_row[order]
    g_rel = rel[order]
    counts = np.bincount(key[order], minlength=NCORES * NBLK * NQ)
    counts = counts.reshape(NCORES, NBLK, NQ)

    Tq = np.maximum(cdiv_arr(counts.max(axis=0), 128), 1)   # [NBLK, NQ]

    starts = np.zeros(NCORES * NBLK * NQ + 1, np.int64)
    np.cumsum(counts.reshape(-1), out=starts[1:])

    per_core = []
    for c in range(NCORES):
        streams = []
        for qi in range(NQ):
            idx_list = []
            rel_list = []
            for b in range(NBLK):
                gi = (c * NBLK + b) * NQ + qi
                s, e = starts[gi], starts[gi + 1]
                n = e - s
                cap = Tq[b, qi] * 128
                assert n <= cap
                pad = cap - n
                idx_list.append(
                    np.concatenate([g_row[s:e], np.zeros(pad, np.int64)]))
                rel_list.append(
                    np.concatenate([g_rel[s:e] - b * 128,
                                    np.full(pad, -1, np.int64)]))
            streams.append((np.concatenate(idx_list), np.concatenate(rel_list)))
        sidx = np.arange(RPC_PAD, dtype=np.int64)
        self_rel = np.where(sidx < RPC, sidx % 128, -1)
        per_core.append((streams, self_rel))

    # ---- stage-4: scalar SpMV tiles (incl. self loops as ordinary edges) ---
    c4 = dst_all // RPC
    pid = (src_all // RPC) * RPC_PAD + (src_all % RPC)   # padded id
    win = pid >> 7
    gkey4 = c4 * NWIN + win
    order4 = np.argsort(gkey4, kind="stable")
    gs4 = gkey4[order4]
    cnt4 = np.bincount(gkey4, minlength=NCORES * NWIN).reshape(NCORES, NWIN)
    Tw = cdiv_arr(cnt4.max(axis=0), 128)             # [NWIN], all >= 1
    T4 = int(Tw.sum())
    wtilebase = np.zeros(NWIN + 1, np.int64)
    np.cumsum(Tw, out=wtilebase[1:])

    gstart = np.zeros(NCORES * NWIN + 1, np.int64)
    np.cumsum(cnt4.reshape(-1), out=gstart[1:])
    pos = np.arange(len(order4)) - gstart[gs4]
    slot = (wtilebase[win[order4]] + (pos >> 7)) * 128 + (pos & 127)

    so_src = pid[order4] & 127                       # window offset
    so_dmod = (dst_all[order4] % RPC) & 127
    so_dblk = (dst_all[order4] % RPC) >> 7
    so_core = c4[order4]

    s4_per_core = []
    nslots = T4 * 128
    for c in range(NCORES):
        m = so_core == c
        sl = slot[m]
        woff = np.full(nslots, -1, np.int16)
        dmod = np.full(nslots, -1, np.int16)
        dblk = np.full(nslots, -1, np.int16)
        woff[sl] = so_src[m]
        dmod[sl] = so_dmod[m]
        dblk[sl] = so_dblk[m]
        s4_per_core.append((woff, dmod, dblk))

    return dict(dinv=dinv, sqdeg=sqdeg, Tq=Tq, per_core=per_core,
                Tw=Tw, T4=T4, s4_per_core=s4_per_core)


def _chunk_list(n_tiles):
    out = []
    t = 0
    while t < n_tiles:
        ct = min(CHUNK_TILES, n_tiles - t)
        out.append((t, ct))
        t += ct
    return out


def _wrap_idx(idx_stream, chunks):
    """int16 idx array [128, sum(8*ct)], each chunk 16-wrapped separately."""
    cols = []
    for (t0, ct) in chunks:
        seg = idx_stream[t0 * 128:(t0 + ct) * 128].astype(np.int16)
        w = seg.reshape(-1, 16).T.copy()          # [16, ct*8]
        cols.append(np.tile(w, (8, 1)))           # [128, ct*8]
    return np.concatenate(cols, axis=1)


def _ind_tiles(rel_stream):
    """fp8 indicator array [128, T*128]; tile t cols t*128:(t+1)*128."""
    T = len(rel_stream) // 128
    on = (rel_stream[:, None] == np.arange(128)[None, :])
    on = on.reshape(T, 128, 128).transpose(1, 0, 2).reshape(128, T * 128)
    return on.astype(F8)


def _build_program(Tq, Tw, sim_single_core=False, ablate=None,
                   zero_bias=False):
    """ablate: None | 'noag' | 'gathers' (only DMA streams) | 'mm'."""
    TQ = [int(Tq[:, qi].sum()) for qi in range(NQ)]      # tiles per pass
    q_chunks = [_chunk_list(t) for t in TQ]
    idx_cols = 8 * (sum(TQ) + NBLK)
    ind_tile_base = np.concatenate([[0], np.cumsum(TQ)])  # into indq_d
    T4 = int(Tw.sum())
    win_of = np.repeat(np.arange(NWIN), Tw).tolist()
    if sim_single_core:
        ablate = "noag"
    no_collectives = ablate is not None
    do_mm = ablate in (None, "noag", "mm")
    do_gather = ablate in (None, "noag", "gathers")

    nc = bacc.Bacc("TRN2", target_bir_lowering=False, debug=False,
                   num_devices=1 if sim_single_core else NCORES,
                   num_swdge_queues=NQUEUES)

    dt = mybir.dt
    xT_d = nc.dram_tensor("xT", [CH, RPC_PAD], dt.bfloat16, kind="ExternalInput")
    w1_d = nc.dram_tensor("w1", [CH, CH], dt.bfloat16, kind="ExternalInput")
    wt_d = nc.dram_tensor("wt", [2, CH], dt.float32, kind="ExternalInput")
    w2f_d = nc.dram_tensor("w2f", [1, CH], dt.float32, kind="ExternalInput")
    wf2_d = nc.dram_tensor("wf2", [1, CH], dt.float32, kind="ExternalInput")
    b1_d = nc.dram_tensor("b1", [1, CH], dt.float32, kind="ExternalInput")
    bt_d = nc.dram_tensor("bt", [1, CH], dt.float32, kind="ExternalInput")
    bc_d = nc.dram_tensor("bconst", [1, 1], dt.float32, kind="ExternalInput")
    tfT_d = nc.dram_tensor("tfT", [2, RPC_PAD], dt.float32, kind="ExternalInput")
    dinv_d = nc.dram_tensor("dinvc", [128, NBLK], dt.float32, kind="ExternalInput")
    sqdeg_d = nc.dram_tensor("sqdegc", [1, RPC_PAD], dt.float32, kind="ExternalInput")
    idx_d = nc.dram_tensor("idx", [128, idx_cols], dt.int16, kind="ExternalInput")
    indq_d = nc.dram_tensor("indq", [128, sum(TQ) * 128], dt.float8e4,
                            kind="ExternalInput")
    indself_d = nc.dram_tensor("indself", [128, NBLK * 128], dt.float8e4, kind="ExternalInput")
    sel4_d = nc.dram_tensor("sel4", [128, T4 * 128], dt.float8e4, kind="ExternalInput")
    ind4_d = nc.dram_tensor("ind4", [128, T4 * 128], dt.float8e4, kind="ExternalInput")
    bm4_d = nc.dram_tensor("bm4", [128, T4 * NBLK], dt.float8e4, kind="ExternalInput")
    out_d = nc.dram_tensor("out", [128, NBLK], dt.float32, kind="ExternalOutput")

    rows_of = lambda b: min(128, RPC - b * 128)
    blk_q = np.searchsorted(np.asarray(QBLK), np.arange(NBLK), side="right") - 1

    with tile.TileContext(nc) as tc:
        with tc.tile_pool(name="sbuf", bufs=1) as sb, \
             tc.tile_pool(name="psum", bufs=1, space="PSUM") as ps, \
             tc.tile_pool(name="dram", bufs=1, space="DRAM") as dr:
            nc.gpsimd.load_library(library_config.mlp)

            # ---- resident constants ----
            idx_sb = sb.tile([128, idx_cols], dt.int16)
            nc.sync.dma_start(idx_sb[:], idx_d[:])
            indself_sb = sb.tile([128, NBLK * 128], dt.float8e4)
            nc.sync.dma_start(indself_sb[:], indself_d[:])
            w1_sb = sb.tile([128, 2, CH], dt.bfloat16)
            for k in range(2):
                nc.sync.dma_start(w1_sb[:, k, :], w1_d[k * 128:(k + 1) * 128, :])
            wt_sb = sb.tile([2, CH], dt.float32)
            nc.sync.dma_start(wt_sb[:], wt_d[:])
            w2fb = sb.tile([128, CH], dt.float32)
            nc.sync.dma_start(w2fb[:], w2f_d[:].to_broadcast([128, CH]))
            wf2b = sb.tile([128, CH], dt.float32)
            nc.sync.dma_start(wf2b[:], wf2_d[:].to_broadcast([128, CH]))
            b1_sb = sb.tile([1, CH], dt.float32)
            nc.sync.dma_start(b1_sb[:], b1_d[:])
            bt_sb = sb.tile([1, CH], dt.float32)
            nc.sync.dma_start(bt_sb[:], bt_d[:])
            bc_sb = sb.tile([128, 1], dt.float32)
            nc.sync.dma_start(bc_sb[:], bc_d[:].to_broadcast([128, 1]))
            dinv_sb = sb.tile([128, NBLK], dt.float32)
            nc.sync.dma_start(dinv_sb[:], dinv_d[:])
            ones_row = sb.tile([1, 128], dt.float32)
            nc.vector.memset(ones_row[:], 1.0)

            xtc = sb.tile([128, NBLK], dt.float32)   # x_time . wf2 per (p, b)
            zbuf = sb.tile([128, NBLK], dt.float16)  # z = dinv * (h . w2f)

            def sqdeg_row(b):
                sq = sb.tile([1, 128], dt.float32, tag="sqrow", bufs=3,
                             name=f"sq_{b}")
                nc.sync.dma_start(sq[:], sqdeg_d[:, b * 128:(b + 1) * 128])
                return sq

            # ---- AllGather buffers (per quarter) ----
            qsz = [QROW[qi + 1] - QROW[qi] for qi in range(NQ)]
            ag_in = [dr.tile([qsz[qi], CH], dt.bfloat16, name=f"ag_in{qi}",
                             tag=f"ag_in{qi}") for qi in range(NQ)]
            zin = dr.tile([128, NBLK], dt.float16, name="zin", tag="zin")
            if no_collectives:
                ag_out = [dr.tile([NCORES * qsz[qi], CH], dt.bfloat16,
                                  name=f"tbl{qi}", tag=f"tbl{qi}")
                          for qi in range(NQ)]
                zout = dr.tile([NCORES * 128, NBLK], dt.float16,
                               name="ztbl", tag="ztbl")
            else:
                ag_out = [dr.tile([NCORES * qsz[qi], CH], dt.bfloat16,
                                  addr_space="Shared", name=f"ag_out{qi}",
                                  tag=f"ag_out{qi}") for qi in range(NQ)]
                zout = dr.tile([NCORES * 128, NBLK], dt.float16,
                               addr_space="Shared", name="zout", tag="zout")

            # ---- stage 1: xws = dinv * (x @ W1) -> ag_in_q; AG-q asap ----
            def emit_ag(qi):
                if not no_collectives:
                    nc.gpsimd.collective_compute(
                        "AllGather", mybir.AluOpType.bypass,
                        replica_groups=[list(range(NCORES))],
                        ins=[ag_in[qi][:]], outs=[ag_out[qi][:]])

            if do_mm:
                for b in range(NBLK):
                    rows = rows_of(b)
                    qi = int(blk_q[b])
                    r0 = b * 128 - QROW[qi]
                    xwp = ps.tile([128, CH], dt.float32, space="PSUM",
                                  tag="accps", bufs=4, name=f"xwp_{b}")
                    for k in range(2):
                        xT = sb.tile([128, 128], dt.bfloat16, tag="xT", bufs=4,
                                     name=f"xT_{b}_{k}")
                        nc.sync.dma_start(
                            xT[:], xT_d[k * 128:(k + 1) * 128,
                                        b * 128:(b + 1) * 128])
                        nc.tensor.matmul(xwp[:], lhsT=xT[:], rhs=w1_sb[:, k, :],
                                         start=(k == 0), stop=(k == 1))
                    xws = sb.tile([128, CH], dt.bfloat16, tag="xws", bufs=3,
                                  name=f"xws_{b}")
                    nc.scalar.activation(xws[:], xwp[:],
                                         mybir.ActivationFunctionType.Copy,
                                         scale=dinv_sb[:, b:b + 1])
                    nc.sync.dma_start(ag_in[qi][r0:r0 + rows, :], xws[:rows, :])
                    if b == QBLK[qi + 1] - 1:
                        emit_ag(qi)
                # x_time: fills the AG shadow
                for b in range(NBLK):
                    tfb = sb.tile([2, 8 * 128], dt.float32, tag="tfb", bufs=2,
                                  name=f"tfb_{b // 8}") if b % 8 == 0 else tfb
                    if b % 8 == 0:
                        c0 = b * 128
                        cw = min(8 * 128, RPC_PAD - c0)
                        nc.sync.dma_start(tfb[:, :cw], tfT_d[:, c0:c0 + cw])
                    pt = ps.tile([128, CH], dt.float32, space="PSUM", tag="tp",
                                 bufs=2, name=f"pt_{b}")
                    nc.tensor.matmul(pt[:], lhsT=tfb[:, (b % 8) * 128:(b % 8 + 1) * 128],
                                     rhs=wt_sb[:], start=True,
                                     stop=zero_bias)
                    if not zero_bias:
                        nc.tensor.matmul(pt[:], lhsT=ones_row[:], rhs=bt_sb[:],
                                         start=False, stop=True)
                    xt = sb.tile([128, CH], dt.float32, tag="xt", bufs=2,
                                 name=f"xt_{b}")
                    nc.scalar.activation(xt[:], pt[:],
                                         mybir.ActivationFunctionType.Relu)
                    xp = sb.tile([128, CH], dt.float32, tag="xp", bufs=2,
                                 name=f"xp_{b}")
                    nc.vector.tensor_tensor(out=xp[:], in0=xt[:], in1=wf2b[:],
                                            op=mybir.AluOpType.mult)
                    nc.vector.tensor_reduce(out=xtc[:, b:b + 1], in_=xp[:],
                                            op=mybir.AluOpType.add,
                                            axis=mybir.AxisListType.X)
            else:
                for qi in range(NQ):
                    emit_ag(qi)

            # ---- L1 scatter: 4 passes, one per quarter table ----
            qctr = [0]

            class Stream:
                """Lazy chunked gather stream: tile index -> (rhs, ind) APs."""

                def __init__(self, qi):
                    self.chunks = q_chunks[qi]
                    self.colbase = 8 * int(ind_tile_base[qi])
                    self.ind_base = int(ind_tile_base[qi])
                    self.src_ap = ag_out[qi][:]
                    self.ci = -1
                    self.cursor = 0
                    self.qi = qi

                def _fetch(self, ci):
                    t0, ct = self.chunks[ci]
                    g = sb.tile([128, CHUNK_TILES, CH], dt.bfloat16, tag="gq",
                                bufs=4, name=f"g{self.qi}_{ci}")
                    nidx = ct * 128
                    colb = self.colbase + 8 * t0
                    if do_gather:
                        nc.gpsimd.dma_gather(g[:, :ct, :], self.src_ap,
                                             idx_sb[:, colb:colb + 8 * ct],
                                             nidx, nidx, CH,
                                             queue_num=qctr[0] % NQUEUES)
                        qctr[0] += 1
                    else:
                        nc.gpsimd.memset(g[:, 0, 0:1], 0)
                    self.g = g
                    ind = sb.tile([128, CHUNK_TILES * 128], dt.float8e4,
                                  tag="iq", bufs=4, name=f"i{self.qi}_{ci}")
                    if do_gather:
                        ib = (self.ind_base + t0) * 128
                        nc.scalar.dma_start(ind[:, :ct * 128],
                                            indq_d[:, ib:ib + ct * 128])
                    else:
                        nc.gpsimd.memset(ind[:, 0:1], 0)
                    self.ind = ind
                    self.ci = ci
                    self.t0 = t0

                def next_tile(self):
                    cur = self.cursor
                    ci = cur // CHUNK_TILES
                    if ci != self.ci:
                        self._fetch(ci)
                    slot = cur - self.t0
                    self.cursor = cur + 1
                    return self.g[:, slot, :], self.ind[:, slot * 128:(slot + 1) * 128]

            stash = [None] * NBLK

            def scatter_pass(qi):
                st = Stream(qi)
                last = qi == NQ - 1
                if last:
                    # self-loop stream from own quarter shards
                    gself = sb.tile([128, NBLK, CH], dt.bfloat16, tag="gself",
                                    bufs=1, name="gself")
                    if do_gather:
                        nc.vector.memset(gself[:, NBLK - 1, :], 0.0)
                        for qj in range(NQ):
                            b0, b1 = QBLK[qj], QBLK[qj + 1]
                            full = (min(b1 * 128, RPC) - b0 * 128) // 128 * 128
                            nc.sync.dma_start(
                                gself[:, b0:b0 + full // 128, :],
                                ag_in[qj][:full, :].rearrange(
                                    "(b p) c -> p b c", p=128))
                        lastrows = RPC - (NBLK - 1) * 128
                        nc.sync.dma_start(
                            gself[:lastrows, NBLK - 1, :],
                            ag_in[NQ - 1][(NBLK - 1) * 128 - QROW[NQ - 1]:, :])
                    else:
                        nc.gpsimd.memset(gself[:, 0, 0:1], 0)

                for b in range(NBLK):
                    first = [True]
                    if do_mm:
                        agg = ps.tile([128, CH], dt.float32, space="PSUM",
                                      tag="accps", bufs=4, name=f"agg{qi}_{b}")
                        if qi == 0 and not zero_bias:
                            sq = sqdeg_row(b)
                            nc.tensor.matmul(agg[:], lhsT=sq[:],
                                             rhs=b1_sb[:], start=True, stop=False)
                            first[0] = False
                        if qi > 0:
                            # merge stashed partial back via identity matmul
                            nc.tensor.matmul(
                                agg[:], lhsT=indself_sb[:, b * 128:(b + 1) * 128],
                                rhs=stash[b][:], start=True, stop=False)
                            first[0] = False

                    def mm(ind, rhs, stop=False):
                        nc.tensor.matmul(agg[:], lhsT=ind, rhs=rhs,
                                         start=first[0], stop=stop)
                        first[0] = False

                    nt = int(Tq[b, qi])
                    for i in range(nt):
                        rhs, ind = st.next_tile()
                        if do_mm:
                            mm(ind, rhs, stop=(not last and i == nt - 1))
                    if not do_mm:
                        continue
                    if last:
                        mm(indself_sb[:, b * 128:(b + 1) * 128], gself[:, b, :],
                           stop=True)
                        # epilogue: h = relu(dinv*agg); z = dinv*(h . w2f)
                        h = sb.tile([128, CH], dt.float32, tag="h", bufs=2,
                                    name=f"h_{b}")
                        nc.scalar.activation(h[:], agg[:],
                                             mybir.ActivationFunctionType.Relu,
                                             scale=dinv_sb[:, b:b + 1])
                        pr = sb.tile([128, CH], dt.float32, tag="pr", bufs=2,
                                     name=f"pr_{b}")
                        nc.vector.tensor_tensor(out=pr[:], in0=h[:], in1=w2fb[:],
                                                op=mybir.AluOpType.mult)
                        zr = sb.tile([128, 1], dt.float32, tag="zr", bufs=2,
                                     name=f"zr_{b}")
                        nc.vector.tensor_reduce(out=zr[:], in_=pr[:],
                                                op=mybir.AluOpType.add,
                                                axis=mybir.AxisListType.X)
                        nc.vector.tensor_tensor(out=zbuf[:, b:b + 1], in0=zr[:],
                                                in1=dinv_sb[:, b:b + 1],
                                                op=mybir.AluOpType.mult)
                    else:
                        s_t = sb.tile([128, CH], dt.float16, tag="stash",
                                      bufs=NBLK, name=f"st{qi}_{b}")
                        nc.scalar.activation(s_t[:], agg[:],
                                             mybir.ActivationFunctionType.Copy)
                        stash[b] = s_t

            for qi in range(NQ):
                scatter_pass(qi)

            # ---- z AllGather ----
            if do_mm:
                nc.sync.dma_start(zin[:], zbuf[:])
            if not no_collectives:
                nc.gpsimd.collective_compute(
                    "AllGather", mybir.AluOpType.bypass,
                    replica_groups=[list(range(NCORES))],
                    ins=[zin[:]], outs=[zout[:]])
            ztab = sb.tile([128, NCORES, NBLK], dt.float16)
            nc.sync.dma_start(ztab[:],
                              zout[:].rearrange("(c p) b -> p c b", p=128))

            # ---- stage 4: scalar SpMV via SEL/inject/IND matmul pipeline ----
            out4 = ps.tile([128, NBLK], dt.float32, space="PSUM", tag="out4",
                           bufs=1)

            def emit_inds(prev):
                g0, gn, ind_sb4, inj = prev
                for j in range(gn):
                    t = g0 + j
                    nc.tensor.matmul(out4[:], lhsT=ind_sb4[:, j * 128:(j + 1) * 128],
                                     rhs=inj[:, j, :], start=(t == 0),
                                     stop=(t == T4 - 1))

            prev = None
            if do_mm or do_gather:
                for g0 in range(0, T4, SC):
                    gn = min(SC, T4 - g0)
                    gi = g0 // SC
                    sel_sb = sb.tile([128, SC * 128], dt.float8e4, tag="sel4",
                                     bufs=3, name=f"sel_{gi}")
                    ind_sb4 = sb.tile([128, SC * 128], dt.float8e4, tag="ind4",
                                      bufs=3, name=f"i4_{gi}")
                    bm_sb = sb.tile([128, SC, NBLK], dt.float8e4, tag="bm4",
                                    bufs=3, name=f"bm_{gi}")
                    if do_gather:
                        nc.sync.dma_start(sel_sb[:, :gn * 128],
                                          sel4_d[:, g0 * 128:(g0 + gn) * 128])
                        nc.scalar.dma_start(ind_sb4[:, :gn * 128],
                                            ind4_d[:, g0 * 128:(g0 + gn) * 128])
                        nc.scalar.dma_start(
                            bm_sb[:, :gn, :],
                            bm4_d[:, g0 * NBLK:(g0 + gn) * NBLK].rearrange(
                                "p (s b) -> p s b", s=gn))
                    else:
                        nc.gpsimd.memset(sel_sb[:, 0:1], 0)
                        nc.gpsimd.memset(ind_sb4[:, 0:1], 0)
                        nc.gpsimd.memset(bm_sb[:, 0, 0:1], 0)
                    if do_mm:
                        vp = ps.tile([128, CH], dt.float32, space="PSUM",
                                     tag="tp", bufs=2, name=f"vp_{gi}")
                        for j in range(gn):
                            w = win_of[g0 + j]
                            nc.tensor.matmul(
                                vp[:, j:j + 1],
                                lhsT=sel_sb[:, j * 128:(j + 1) * 128],
                                rhs=ztab[:, w // NBLK, (w % NBLK):(w % NBLK) + 1],
                                start=True, stop=True)
                        vs = sb.tile([128, SC], dt.float32, tag="vs", bufs=2,
                                     name=f"vs_{gi}")
                        nc.vector.tensor_copy(vs[:, :gn], vp[:, :gn])
                        inj = sb.tile([128, SC, NBLK], dt.bfloat16, tag="inj",
                                      bufs=2, name=f"inj_{gi}")
                        vsb = vs[:, :gn].rearrange("p (s o) -> p s o", o=1) \
                                        .to_broadcast([128, gn, NBLK])
                        nc.vector.tensor_tensor(out=inj[:, :gn, :],
                                                in0=bm_sb[:, :gn, :], in1=vsb,
                                                op=mybir.AluOpType.mult)
                        if prev is not None:
                            emit_inds(prev)
                        prev = (g0, gn, ind_sb4, inj)
            if prev is not None:
                emit_inds(prev)

            # ---- head: out = dinv*OUT4 + xtc (+ const) ----
            res = sb.tile([128, NBLK], dt.float32)
            if do_mm:
                nc.vector.tensor_tensor(out=res[:], in0=out4[:], in1=dinv_sb[:],
                                        op=mybir.AluOpType.mult)
                nc.vector.tensor_tensor(out=res[:], in0=res[:], in1=xtc[:],
                                        op=mybir.AluOpType.add)
                if not zero_bias:
                    nc.vector.tensor_tensor(
                        out=res[:], in0=res[:],
                        in1=bc_sb[:].to_broadcast([128, NBLK]),
                        op=mybir.AluOpType.add)
            else:
                nc.vector.memset(res[:], 0.0)
            nc.sync.dma_start(out_d[:], res[:])

    nc.compile()
    return nc


def _host_inputs(inputs, prep):
    x = np.asarray(inputs["x"], np.float32)
    tf = np.asarray(inputs["time_features"], np.float32)
    W1 = np.asarray(inputs["W1"], np.float32)
    W2 = np.asarray(inputs["W2"], np.float32)
    Wt = np.asarray(inputs["Wt"], np.float32)
    Wf = np.asarray(inputs["Wf"], np.float32).reshape(-1)
    b1 = np.asarray(inputs["b1"], np.float32)
    b2 = np.asarray(inputs["b2"], np.float32)
    bt = np.asarray(inputs["bt"], np.float32)
    bf = np.asarray(inputs["bf"], np.float32).reshape(-1)

    dinv, sqdeg = prep["dinv"], prep["sqdeg"]
    Tq = prep["Tq"]
    per_core = prep["per_core"]
    T4 = prep["T4"]

    w2f = (W2 @ Wf[:CH]).reshape(1, CH).astype(np.float32)
    wf2 = Wf[CH:].reshape(1, CH).astype(np.float32)
    bconst = np.array([[float(b2 @ Wf[:CH] + bf[0])]], np.float32)

    TQ = [int(Tq[:, qi].sum()) for qi in range(NQ)]
    q_chunks = [_chunk_list(t) for t in TQ]

    ar128 = np.arange(128, dtype=np.int16)
    arblk = np.arange(NBLK, dtype=np.int16)

    in_maps = []
    for c in range(NCORES):
        streams, self_rel = per_core[c]
        idx = np.concatenate(
            [_wrap_idx(streams[qi][0], q_chunks[qi]) for qi in range(NQ)]
            + [_wrap_idx(np.where(np.arange(RPC_PAD) < RPC,
                                  np.arange(RPC_PAD), 0),
                         _chunk_list(NBLK))], axis=1)
        indq = np.concatenate(
            [_ind_tiles(streams[qi][1]) for qi in range(NQ)], axis=1)
        r0 = c * RPC
        tfT = np.zeros((2, RPC_PAD), np.float32)
        tfT[:, :RPC] = tf[r0:r0 + RPC].T
        dv = np.zeros(RPC_PAD, np.float32)
        dv[:RPC] = dinv[r0:r0 + RPC]
        dinv_c = dv.reshape(NBLK, 128).T.copy()
        sq_c = np.zeros((1, RPC_PAD), np.float32)
        sq_c[0, :RPC] = sqdeg[r0:r0 + RPC]
        xT = np.zeros((CH, RPC_PAD), BF16)
        xT[:, :RPC] = x[r0:r0 + RPC].T.astype(BF16)

        woff, dmod, dblk = prep["s4_per_core"][c]
        wo = woff.reshape(T4, 128)
        sel4 = (ar128[:, None, None] == wo[None, :, :]).reshape(128, T4 * 128)
        dm = dmod.reshape(T4, 128)
        ind4 = (dm[:, :, None] == ar128[None, None, :]) \
            .transpose(1, 0, 2).reshape(128, T4 * 128)
        db = dblk.reshape(T4, 128)
        bm4 = (db[:, :, None] == arblk[None, None, :]) \
            .transpose(1, 0, 2).reshape(128, T4 * NBLK)

        in_maps.append({
            "xT": xT,
            "w1": W1.astype(BF16),
            "wt": Wt,
            "w2f": w2f, "wf2": wf2,
            "b1": b1[None, :], "bt": bt[None, :], "bconst": bconst,
            "tfT": tfT,
            "dinvc": dinv_c,
            "sqdegc": sq_c,
            "idx": idx,
            "indq": indq,
            "indself": _ind_tiles(self_rel),
            "sel4": sel4.astype(F8),
            "ind4": ind4.astype(F8),
            "bm4": bm4.astype(F8),
        })
    return in_maps


def _run(inputs, trace=False):
    prep = _preprocess(inputs["edge_index"])
    zero_bias = not any(np.any(np.asarray(inputs[k]))
                        for k in ("b1", "b2", "bt", "bf"))
    key = (tuple(prep["Tq"].reshape(-1).tolist()),
           tuple(prep["Tw"].tolist()), zero_bias)
    if key not in _cache:
        _cache.clear()
        _cache[key] = _build_program(prep["Tq"], prep["Tw"],
                                     zero_bias=zero_bias)
    nc = _cache[key]
    in_maps = _host_inputs(inputs, prep)
    res = run_bass_kernel_spmd(nc, in_maps, core_ids=list(range(NCORES)),
                               trace=trace)
    out = np.concatenate(
        [np.asarray(res.results[c]["out"]).T.reshape(-1, 1)[:RPC]
         for c in range(NCORES)], axis=0)
    return out.astype(np.float32), res


def kernel(**inputs):
    out, _ = _run(inputs, trace=False)
    return out
